# revision 1
# baseline (speedup 1.0000x reference)
"""Correlation-network kernel for TRN2, batch-sharded over 8 NeuronCores.

Per core (one batch element b):
  A = feature_A[b] as [HW=2304, C=256], B = feature_B[b] likewise.
  out[m, n] = corr_raw[m, n] * s[n]
  where corr_raw = A @ B^T  and  s[n] = 1/sqrt(sum_m corr_raw[m, n]^2).
  The 1/C of the reference cancels between corr and penalty.

Key algebra: sum_m corr_raw[m,n]^2 = b_n^T (A^T A) b_n, so the column norms
come from a tiny Gram-matrix chain (G = A^T A; Q = G B^T; pen2 = colsum(B^T.Q))
instead of a second pass over the [2304,2304] output. The normalization is
folded into the GEMM by pre-scaling B^T's columns.

Matmuls run in float32r (full-rate on the PE array vs 4x slower fp32).
Main loop is column-chunk-outer so the first output DMA starts as soon as the
first chunk's scale s[n0:n0+cw] is ready; the output stream (~60us, the
binding resource) then runs back-to-back.
"""
import numpy as np

B, H, W, C = 8, 48, 48, 256
HW = H * W            # 2304
MT = HW // 128        # 18 m-tiles
CHUNKS = [(0, 512), (512, 512), (1024, 512), (1536, 512), (2048, 256)]

_CACHE = {}


def _build(reps=1):
    import concourse.bacc as bacc
    import concourse.mybir as mybir
    import concourse.tile as tile
    from concourse.masks import make_identity

    dt = mybir.dt
    f32 = dt.float32
    f32r = dt.float32r

    nc = bacc.Bacc(None, target_bir_lowering=False, debug=False)
    a_dram = nc.dram_tensor("a", [HW, C], f32r, kind="ExternalInput")
    b_dram = nc.dram_tensor("b", [HW, C], f32r, kind="ExternalInput")
    o_dram = nc.dram_tensor("out", [HW, HW], f32, kind="ExternalOutput")

    a_r = a_dram[:, :].rearrange("(t p) c -> p t c", p=128)
    b_r = b_dram[:, :].rearrange("(t p) c -> p t c", p=128)
    o_r = o_dram[:, :].rearrange("(t p) n -> p t n", p=128)

    with tile.TileContext(nc) as tc:
      for _rep in range(reps):
        with tc.tile_pool(name="singles", bufs=1) as sb:
            id_f = sb.tile([128, 128], f32)
            make_identity(nc, id_f)
            ident = sb.tile([128, 128], f32r)
            nc.vector.tensor_copy(ident, id_f)

            at = [sb.tile([128, HW], f32r, tag=f"at{h}", name=f"at{h}")
                  for h in (0, 1)]
            bt = [sb.tile([128, HW], f32r, tag=f"bt{h}", name=f"bt{h}")
                  for h in (0, 1)]
            g_sb = [sb.tile([128, C], f32r, tag=f"g{h}", name=f"g{h}")
                    for h in (0, 1)]
            r_sb = [sb.tile([128, HW], f32r, tag=f"r{h}", name=f"r{h}")
                    for h in (0, 1)]
            ones_f = sb.tile([128, 1], f32)
            nc.vector.memset(ones_f, 1.0)
            ones = sb.tile([128, 1], f32r)
            nc.vector.tensor_copy(ones, ones_f)
            ones1_f = sb.tile([1, 128], f32)
            nc.vector.memset(ones1_f, 1.0)

            # ---- phase 1: load, transpose, Gram ----
            natp = tc.alloc_tile_pool(name="nat", bufs=1)
            ps_a = tc.alloc_tile_pool(name="ps_a", bufs=1, space="PSUM")
            if True:
                b_nat = natp.tile([128, MT, C], f32r)
                a_nat = natp.tile([128, MT, C], f32r)
                pen2 = natp.tile([1, HW], f32)
                s_sb = natp.tile([1, HW], f32)
                for q0, q1 in ((0, 4), (4, 9), (9, 13), (13, MT)):
                    nc.sync.dma_start(out=b_nat[:, q0:q1], in_=b_r[:, q0:q1])
                half = MT // 2
                nc.sync.dma_start(out=a_nat[:, :half], in_=a_r[:, :half])
                nc.sync.dma_start(out=a_nat[:, half:], in_=a_r[:, half:])

                def transpose_op(src, dst, t, h, i):
                    pt = ps_a.tile([128, 128], f32r, tag="pt", name="pt",
                                   bufs=2)
                    nc.tensor.transpose(
                        pt, src[:, t, h * 128:(h + 1) * 128], ident)
                    cp = (nc.vector.tensor_copy if i % 2 == 0
                          else nc.scalar.copy)
                    cp(dst[h][:, t * 128:(t + 1) * 128], pt)

                i = 0
                for t in range(MT):
                    for h in (0, 1):
                        transpose_op(b_nat, bt, t, h, i)
                        i += 1
                for h in (0, 1):
                    pg = ps_a.tile([128, C], f32, tag="pg", name="pg", bufs=1)
                    for t in range(MT):
                        nc.tensor.matmul(
                            pg, a_nat[:, t, h * 128:(h + 1) * 128],
                            a_nat[:, t, :], start=(t == 0), stop=(t == MT - 1))
                    nc.vector.tensor_copy(g_sb[h], pg)
                for t in range(3):
                    for h in (0, 1):
                        transpose_op(a_nat, at, t, h, i)
                        i += 1
            # ---- phase 2: per-chunk scale pipeline; the first 3 m-tiles'
            # GEMM is interleaved per chunk so their row panels (and the
            # output DMA stream) complete with the s-pipe ----
            panels = tc.alloc_tile_pool(name="panels", bufs=5)
            early = [panels.tile([128, HW], f32, tag="panel",
                                 name=f"panel{mt}") for mt in range(3)]
            with tc.tile_pool(name="ps_s", bufs=1, space="PSUM") as ps_s:
                for ci, (n0, cw) in enumerate(CHUNKS):
                    for h2 in (0, 1):
                        pq = ps_s.tile([128, 512], f32, tag="pq", name="pq",
                                       bufs=1)
                        for h in (0, 1):
                            nc.tensor.matmul(
                                pq[:, :cw],
                                g_sb[h][:, h2 * 128:(h2 + 1) * 128],
                                bt[h][:, n0:n0 + cw],
                                start=(h == 0), stop=(h == 1))
                        nc.vector.tensor_mul(
                            r_sb[h2][:, n0:n0 + cw], bt[h2][:, n0:n0 + cw],
                            pq[:, :cw])
                    pp = ps_s.tile([1, 512], f32, tag="pp", name="pp", bufs=1)
                    for h2 in (0, 1):
                        nc.tensor.matmul(pp[:, :cw], ones,
                                         r_sb[h2][:, n0:n0 + cw],
                                         start=(h2 == 0), stop=(h2 == 1))
                    nc.scalar.activation(pen2[:, n0:n0 + cw], pp[:, :cw],
                                         mybir.ActivationFunctionType.Sqrt)
                    nc.vector.reciprocal(s_sb[:, n0:n0 + cw],
                                         pen2[:, n0:n0 + cw])
                    pb = ps_s.tile([128, 512], f32, tag="pb", name="pb",
                                   bufs=2)
                    nc.tensor.matmul(pb[:, :cw], ones1_f, s_sb[:, n0:n0 + cw],
                                     start=True, stop=True)
                    # B'^T chunk (reuse R tiles; scale read from PSUM)
                    for h in (0, 1):
                        nc.vector.tensor_mul(r_sb[h][:, n0:n0 + cw],
                                             bt[h][:, n0:n0 + cw],
                                             pb[:, :cw])
                    for mt in range(3):
                        pm = ps_s.tile([128, 512], f32, tag="pm_e",
                                       name="pm_e", bufs=1)
                        for h in (0, 1):
                            nc.tensor.matmul(
                                pm[:, :cw],
                                at[h][:, mt * 128:(mt + 1) * 128],
                                r_sb[h][:, n0:n0 + cw],
                                start=(h == 0), stop=(h == 1))
                        cp = (nc.vector.tensor_copy if mt % 2 == 0
                              else nc.scalar.copy)
                        cp(early[mt][:, n0:n0 + cw], pm[:, :cw])
                    for mt in range(3):
                        nc.sync.dma_start(out=o_r[:, mt, n0:n0 + cw],
                                          in_=early[mt][:, n0:n0 + cw])

            for t in range(3, MT):
                for h in (0, 1):
                    transpose_op(a_nat, at, t, h, i)
                    i += 1
            bts = r_sb

            # ---- phase 3: main GEMM, m-tile outer (row panels) ----
            with tc.tile_pool(name="ps_mm", bufs=5, space="PSUM") as ps_mm:
                for mt in range(3, MT):
                    panel = panels.tile([128, HW], f32, tag="panel",
                                        name="panel")
                    for ci, (n0, cw) in enumerate(CHUNKS):
                        pm = ps_mm.tile([128, 512], f32, tag="pm", name="pm")
                        for h in (0, 1):
                            nc.tensor.matmul(
                                pm[:, :cw],
                                at[h][:, mt * 128:(mt + 1) * 128],
                                bts[h][:, n0:n0 + cw],
                                start=(h == 0), stop=(h == 1))
                        cp = (nc.vector.tensor_copy if ci % 2 == 0
                              else nc.scalar.copy)
                        cp(panel[:, n0:n0 + cw], pm[:, :cw])
                    nc.sync.dma_start(out=o_r[:, mt, :], in_=panel)
            panels.release()
            ps_a.release()
            natp.release()
    nc.finalize()
    return nc


def _get_nc(reps=1):
    key = ("nc", reps)
    if key not in _CACHE:
        _CACHE[key] = _build(reps)
    return _CACHE[key]


def run(feature_A, feature_B, trace=False):
    from concourse.bass_utils import run_bass_kernel_spmd

    nc = _get_nc()
    fa = np.ascontiguousarray(np.asarray(feature_A), dtype=np.float32)
    fb = np.ascontiguousarray(np.asarray(feature_B), dtype=np.float32)
    in_maps = [{"a": fa[i].reshape(HW, C), "b": fb[i].reshape(HW, C)}
               for i in range(B)]
    res = run_bass_kernel_spmd(nc, in_maps, list(range(B)), trace=trace)
    out = np.stack([res.results[i]["out"] for i in range(B)])
    return out.reshape(B, H, W, H, W), res


def kernel(feature_A, feature_B):
    out, _ = run(feature_A, feature_B)
    return out

